# revision 15
# baseline (speedup 1.0000x reference)
"""GAT attention layer (B=8, N=2048, C=512) on 8 TRN2 NeuronCores.

Data-parallel over B: core b handles graph b.
Per-core math (x: [N,C], w: [C,C], a: [2C,1]):
    wa_t = w @ a_t                      (t=0,1)       [C]
    s_t  = x @ wa_t                                   [N]
    p_ji = exp(leaky_relu(s1_i + s2_j))
    r_i  = sum_j p_ji;  out = (p^T @ x) / r

Softmax rows are invariant to any per-row scale, so with
lambda_i = exp(-0.2*s1_i) we compute
    p'_ji = (G_i * F1_j) max F1a_j
    G = exp(0.8*s1), F1 = exp(s2), F1a = exp(0.2*s2)
which is ONE DVE tensor_scalar op per score block (two per-partition
scalars, G broadcast along the free dim) -- no per-element exp at all.

Column->row redistribution (wa, G) uses DVE 32x32 block transposes + a
row-contiguous DRAM hop.  Partition-broadcast DMAs are DESCRIPTOR-bound
(~65ns/partition-row), so each is split across 2-3 DMA rings.  The
critical chain (a,w -> wa -> s1 -> G -> scores) owns the DMA system
early; the bulk x loads are deferred behind it on each ring so their
bandwidth/descriptor load does not stretch the chain's hop latencies.

The score stream is split into i-halves: the first half only needs
s1[0:1024] (8 row-dots); the second half's dots, the x->bf16 casts and
the G half-2 chain all hide inside the stream's DVE slack.

Engine roles:
  PE   : 256 PV MMs (215ns) + 256 r-MMs (25ns)  <- pacer
  DVE  : row-dots (STT+accum), transposes, casts, p' tensor_scalar, recip
  ACT  : G/F exps, half the output normalizes, one DMA ring
  GPS  : DMA ring only (its compute poisons concurrent DVE ~3x)
"""

import sys

import numpy as np

if "/opt/trn_rl_repo" not in sys.path:
    sys.path.insert(0, "/opt/trn_rl_repo")

B, N, C = 8, 2048, 512
P = 128
NJ = N // P  # 16 source-node blocks
NH = N // 2  # 1024: i-half width
ALPHA = 0.2  # leaky_relu slope
# PSUM: 8 banks of [128, 512] fp32. 7 output accumulators + 1 r bank.
GROUPS = [(0, 7), (7, 14), (14, 16)]

_CACHE = {}


def _build():
    from contextlib import ExitStack

    import concourse.bacc as bacc
    import concourse.bass as bass
    import concourse.tile as tile
    from concourse import mybir

    fp32 = mybir.dt.float32
    bf16 = mybir.dt.bfloat16
    AF = mybir.ActivationFunctionType
    OP = mybir.AluOpType

    nc = bacc.Bacc("TRN2", target_bir_lowering=False)
    x_d = nc.dram_tensor("x", [N, C], fp32, kind="ExternalInput")
    w_d = nc.dram_tensor("w", [C, C], fp32, kind="ExternalInput")
    a_d = nc.dram_tensor("a", [2 * C, 1], fp32, kind="ExternalInput")
    o_d = nc.dram_tensor("o", [N, C], fp32, kind="ExternalOutput")

    with ExitStack() as ctx:
        tc = ctx.enter_context(tile.TileContext(nc))
        const = ctx.enter_context(tc.tile_pool(name="const", bufs=1))
        wpool = ctx.enter_context(tc.tile_pool(name="w", bufs=4))
        xpool = ctx.enter_context(tc.tile_pool(name="xin", bufs=NJ))
        xbfp = ctx.enter_context(tc.tile_pool(name="xbf", bufs=NJ))
        ppool = ctx.enter_context(tc.tile_pool(name="p", bufs=2 * NJ))
        scr = ctx.enter_context(tc.tile_pool(name="scr", bufs=6))
        osb = ctx.enter_context(tc.tile_pool(name="osb", bufs=3))
        dram = ctx.enter_context(tc.tile_pool(name="dram", bufs=1, space="DRAM"))
        ps_out = ctx.enter_context(tc.tile_pool(name="ps_out", bufs=7, space="PSUM"))
        ps_r = ctx.enter_context(tc.tile_pool(name="ps_r", bufs=1, space="PSUM"))

        # --- persistent small tiles -------------------------------------
        s1col = const.tile([P, NJ], fp32)  # s1[128j+p] at [p, j]
        s2col = const.tile([P, NJ], fp32)
        F1col = const.tile([P, NJ], fp32)  # exp(s2)
        F1acol = const.tile([P, NJ], fp32)  # exp(ALPHA*s2)
        Gpad = const.tile([P, 32], bf16)  # exp(0.8*s1) cols (j at [:,j])
        GT = const.tile([32, P], bf16)  # half-1 transpose: G[j*128+p] at [j,p]
        GT2 = const.tile([32, P], bf16)  # half-2 transpose
        Gb = const.tile([P, N], bf16)  # G broadcast rows
        wa12 = const.tile([P, 32], fp32)  # wa_t[128q+p] at [p, t*4+q]
        waT1 = const.tile([32, P], fp32)  # rows 0:4 = wa1 chunks
        waT2 = const.tile([32, P], fp32)  # rows 4:8 = wa2 chunks
        abc = const.tile([P, 2, C], fp32)  # a rows broadcast to 128 parts
        wab1 = const.tile([P, C], fp32)  # wa1 row broadcast to 128 parts
        wab2 = const.tile([P, C], fp32)  # wa2 row broadcast to 128 parts
        ones_bf = const.tile([P, 1], bf16)
        ones_f32 = const.tile([P, P], fp32)
        warm_rhs = const.tile([P, C], bf16)
        rinv = const.tile([P, NJ], fp32)
        dummy = const.tile([P, 1], fp32)
        dummy2 = const.tile([P, 1], fp32)
        gda = const.tile([P, 1], fp32)
        gdb = const.tile([P, 1], fp32)

        scratch_wa = dram.tile([2 * C], fp32)
        scratch_G = dram.tile([N], bf16)

        def bcast_sliced(rings, out_fn, src_ap):
            n = len(rings)
            step = P // n
            for i, eng in enumerate(rings):
                lo = i * step
                hi = P if i == n - 1 else lo + step
                eng.dma_start(
                    out=out_fn(lo, hi),
                    in_=bass.AP(
                        tensor=src_ap.tensor,
                        offset=src_ap.offset,
                        ap=[[0, hi - lo]] + list(src_ap.ap),
                    ),
                )

        nc.vector.memset(ones_bf[:], 1.0)
        nc.vector.memset(ones_f32[:], 1.0)
        nc.vector.memset(warm_rhs[:], 0.0)

        # Preload ACT exp table (~2.7us) during the DMA head phase.
        nc.scalar.activation(dummy[:], ones_bf[:], AF.Exp)

        # PE warm-up: back-to-back MMs trip the HAM activity window so the
        # array reaches 8/8 clock before real matmuls arrive.
        warm_ps = ps_r.tile([P, C], fp32, tag="rps", name="warm_ps")
        for _ in range(40):
            nc.tensor.matmul(
                warm_ps[0:1, :],
                lhsT=ones_bf[:],
                rhs=warm_rhs[:],
                start=True,
                stop=True,
                skip_group_check=True,
            )

        def warm_on(rhs_ap):
            nc.tensor.matmul(
                warm_ps[:, :],
                lhsT=ones_f32[:],
                rhs=rhs_ap,
                start=True,
                stop=True,
                skip_group_check=True,
            )

        # --- a -> abc: 2-ring partition broadcast (sync + scalar) --------
        a_rows = a_d[:, 0].rearrange("(t c) -> t c", t=2)  # [2, C]
        bcast_sliced(
            [nc.sync, nc.scalar],
            lambda lo, hi: abc[lo:hi, :, :],
            a_rows,
        )
        # --- w tiles alone on the gpsimd ring ----------------------------
        wt = []
        for q in range(4):
            t = wpool.tile([P, C], fp32, tag="w")
            nc.gpsimd.dma_start(t[:], w_d[q * P : (q + 1) * P, :])
            wt.append(t)

        # --- wa dots on DVE; col->row via block transpose + DRAM hop -----
        def emit_wa_dots(t):
            for q in range(4):
                s = scr.tile([P, C], fp32, tag="ttr")
                nc.vector.scalar_tensor_tensor(
                    out=s[:],
                    in0=wt[q][:],
                    scalar=0.0,
                    in1=abc[:, t, :],
                    op0=OP.add,
                    op1=OP.mult,
                    accum_out=wa12[:, t * 4 + q : t * 4 + q + 1],
                )
            waT = waT1 if t == 0 else waT2
            for b in range(4):
                nc.vector.transpose(waT[0:32, b * 32 : (b + 1) * 32],
                                    wa12[b * 32 : (b + 1) * 32, 0:32])
            nc.gpsimd.dma_start(  # row-contiguous scatter to DRAM
                out=scratch_wa[t * C : (t + 1) * C].rearrange("(r p) -> r p", p=P),
                in_=waT[t * 4 : t * 4 + 4, :],
            )

        xin = [xpool.tile([P, C], fp32, tag="xin", name=f"x_{j}") for j in range(NJ)]
        emit_wa_dots(0)
        for j in range(4, 8):  # DVE reaches here right after waT1
            nc.vector.tensor_copy(gda[0:1, 0:1], xin[j][0:1, 0:1])
        wr1 = scratch_wa[0:C]
        bcast_sliced([nc.gpsimd, nc.scalar], lambda lo, hi: wab1[lo:hi, :], wr1)
        emit_wa_dots(1)
        nc.vector.reciprocal(dummy2[:], dummy[:])  # preload DVE recip table

        # --- x loads, staged to keep the 8-20us DMA window clear for the
        # critical chain: x0-3 immediate; x4-7 released by a DVE gate-read
        # placed after waT1 (~15us); x8-15 by ACT gate-reads after the
        # G half-1 exp (~28us; first consumer is stream slot 0 at ~40us).
        xbf = [xbfp.tile([P, C], bf16, tag="xbf", name=f"xb_{j}") for j in range(NJ)]

        def load_x(j, eng):
            eng.dma_start(xin[j][:], x_d[j * P : (j + 1) * P, :])
            warm_on(xin[j][:])

        for j in range(4):
            load_x(j, nc.scalar if j % 2 == 0 else nc.gpsimd)
        for j in range(4, 8):  # gated by the DVE reads emitted after waT1
            load_x(j, nc.scalar if j % 2 == 0 else nc.gpsimd)
        wr2 = scratch_wa[C : 2 * C]
        bcast_sliced([nc.gpsimd, nc.scalar], lambda lo, hi: wab2[lo:hi, :], wr2)

        # --- s1 half-1 row-dots on DVE (chase the x landings) ------------
        sscr = []

        def emit_s1(j):
            s = scr.tile([P, C], fp32, tag="ttr", name=f"s1scr_{j}")
            nc.vector.scalar_tensor_tensor(
                out=s[:],
                in0=xin[j][:],
                scalar=0.0,
                in1=wab1[:],
                op0=OP.add,
                op1=OP.mult,
                accum_out=s1col[:, j : j + 1],
            )
            return s

        for j in range(8):
            sscr.append(emit_s1(j))

        # half-1 G: exp(0.8*s1[0:1024]) -> transpose -> DRAM -> broadcast
        nc.scalar.activation(Gpad[:, 0:8], s1col[:, 0:8], AF.Exp, scale=1.0 - ALPHA)
        for j in range(8, NJ):  # release the late x loads (ACT gate-reads)
            nc.scalar.activation(gdb[0:1, 0:1], xin[j][0:1, 0:1], AF.Copy)
            load_x(j, (nc.sync, nc.gpsimd, nc.scalar)[j % 3])
        for b in range(4):
            nc.vector.transpose(GT[0:32, b * 32 : (b + 1) * 32],
                                Gpad[b * 32 : (b + 1) * 32, 0:32])
        # casts 0..7 on DVE: fill the G-chain wait window
        for j in range(8):
            nc.vector.tensor_copy(xbf[j][:], xin[j][:])

        # s2 row-dots + F exps (per-block scalars), seeded 2 blocks ahead
        def emit_s2(j):
            s = scr.tile([P, C], fp32, tag="ttr", name=f"s2scr_{j}")
            nc.vector.scalar_tensor_tensor(
                out=s[:],
                in0=xin[j][:],
                scalar=0.0,
                in1=wab2[:],
                op0=OP.add,
                op1=OP.mult,
                accum_out=s2col[:, j : j + 1],
            )
            nc.scalar.activation(F1col[:, j : j + 1], s2col[:, j : j + 1], AF.Exp)
            nc.scalar.activation(
                F1acol[:, j : j + 1], s2col[:, j : j + 1], AF.Exp, scale=ALPHA
            )

        emit_s2(0)
        emit_s2(1)

        nc.sync.dma_start(  # G half-1 scatter on the (idle) sync ring
            out=scratch_G[0:NH].rearrange("(j p) -> j p", p=P),
            in_=GT[0:8, :],
        )
        g1 = scratch_G[0:NH]
        bcast_sliced(
            [nc.sync, nc.gpsimd, nc.scalar],
            lambda lo, hi: Gb[lo:hi, 0:NH],
            g1,
        )

        # PE keep-alives on the s1 scratches (complete ~21-26us)
        for j in (1, 3, 5, 7):
            warm_on(sscr[j][:])
        # fillers: WAW-chained, bridging PE idle until the stream starts
        for _ in range(24):
            nc.tensor.matmul(
                warm_ps[0:1, :],
                lhsT=ones_bf[:],
                rhs=warm_rhs[:],
                start=True,
                stop=True,
                skip_group_check=True,
            )
        nc.tensor.matmul(  # keep-alive blip gated on the Gb broadcast
            warm_ps[0:1, :],
            lhsT=ones_bf[:],
            rhs=Gb[:, 0:C],
            start=True,
            stop=True,
            skip_group_check=True,
        )

        # --- score stream: p'[j,i] = (G_i * F1_j) max F1a_j --------------
        # DVE slot budget (PE pace 2.06us/block): p' 0.55 + s2 0.69
        # + (slots 0-7) s1 half-2 dot 0.69 / (slots 8-15) cast 0.48
        ptA, ptB = [], []
        for j in range(NJ):
            p = ppool.tile([P, NH], bf16, tag="p", name=f"pA_{j}")
            nc.vector.tensor_scalar(
                out=p[:],
                in0=Gb[:, 0:NH],
                scalar1=F1col[:, j : j + 1],
                scalar2=F1acol[:, j : j + 1],
                op0=OP.mult,
                op1=OP.max,
            )
            ptA.append(p)
            if j < 8:  # half-2 s1 dots hide inside the stream
                emit_s1(8 + j)
            else:  # late casts hide in the stream's back half
                nc.vector.tensor_copy(xbf[j][:], xin[j][:])
            if j == 9:  # half-2 G chain (s1 complete by stream slot 8)
                nc.scalar.activation(
                    Gpad[:, 8:16], s1col[:, 8:16], AF.Exp, scale=1.0 - ALPHA
                )
                for b in range(4):
                    nc.vector.transpose(GT2[0:32, b * 32 : (b + 1) * 32],
                                        Gpad[b * 32 : (b + 1) * 32, 0:32])
                nc.sync.dma_start(
                    out=scratch_G[NH:N].rearrange("(j p) -> j p", p=P),
                    in_=GT2[8:16, :],
                )
                g2 = scratch_G[NH:N]
                bcast_sliced(
                    [nc.sync, nc.gpsimd],
                    lambda lo, hi: Gb[lo:hi, NH:N],
                    g2,
                )
            if j + 2 < NJ:
                emit_s2(j + 2)

        # --- half-2 score tiles (feed PSUM groups 1-2) -------------------
        for j in range(NJ):
            p = ppool.tile([P, NH], bf16, tag="p", name=f"pB_{j}")
            nc.vector.tensor_scalar(
                out=p[:],
                in0=Gb[:, NH:N],
                scalar1=F1col[:, j : j + 1],
                scalar2=F1acol[:, j : j + 1],
                op0=OP.mult,
                op1=OP.max,
            )
            ptB.append(p)

        def lhs_chunk(j, k):
            if k < 8:
                return ptA[j][:, k * P : (k + 1) * P]
            return ptB[j][:, (k - 8) * P : (k - 7) * P]

        # --- PV + r + normalize, in PSUM-sized chunk groups --------------
        for g0, g1_ in GROUPS:
            nk = g1_ - g0
            outps = [
                ps_out.tile([P, C], fp32, tag="ops", name=f"ops_{g0}_{ki}")
                for ki in range(nk)
            ]
            rps = ps_r.tile([P, C], fp32, tag="rps")
            for j in range(NJ):
                first, last = j == 0, j == NJ - 1
                for ki, k in enumerate(range(g0, g1_)):
                    lhs = lhs_chunk(j, k)
                    nc.tensor.matmul(
                        outps[ki][:], lhsT=lhs, rhs=xbf[j][:], start=first, stop=last
                    )
                    # start=True clears the WHOLE bank's has_written bits, so
                    # only the very first matmul into this bank may set it;
                    # later first-touches per element overwrite (bit clear)
                    # and the rest accumulate.
                    nc.tensor.matmul(
                        rps[:, ki : ki + 1],
                        lhsT=lhs,
                        rhs=ones_bf[:],
                        start=first and ki == 0,
                        stop=last,
                        skip_group_check=True,
                    )
            nc.vector.reciprocal(rinv[:, g0:g1_], rps[:, :nk])
            for ki, k in enumerate(range(g0, g1_)):
                ob = osb.tile([P, C], fp32, tag="ob")
                if ki % 2 == 0:
                    nc.scalar.activation(
                        ob[:], outps[ki][:], AF.Copy, bias=0.0,
                        scale=rinv[:, k : k + 1],
                    )
                else:
                    nc.vector.tensor_scalar_mul(
                        ob[:], outps[ki][:], rinv[:, k : k + 1]
                    )
                nc.sync.dma_start(o_d[k * P : (k + 1) * P, :], ob[:])

    nc.compile()
    return nc


def _get_nc():
    if "nc" not in _CACHE:
        _CACHE["nc"] = _build()
    return _CACHE["nc"]


def _run(inputs, trace=False, tmpdir=None):
    from concourse.bass_utils import run_bass_kernel_spmd

    nc = _get_nc()
    x = np.ascontiguousarray(np.asarray(inputs["x"], dtype=np.float32))
    w = np.ascontiguousarray(np.asarray(inputs["w"], dtype=np.float32))
    a = np.ascontiguousarray(np.asarray(inputs["a"], dtype=np.float32))
    core_ids = list(range(B))
    in_maps = [{"x": x[b], "w": w, "a": a} for b in core_ids]
    res = run_bass_kernel_spmd(nc, in_maps, core_ids, trace=trace, tmpdir=tmpdir)
    out = np.stack([res.results[b]["o"] for b in core_ids], axis=0)
    return out, res


def kernel(**inputs) -> np.ndarray:
    out, _ = _run(inputs, trace=False)
    return out
